# revision 1
# baseline (speedup 1.0000x reference)
"""NonLocalBlock (single-head attention, N=HW=4096, d=128) on 8 trn2 cores.

Sharding: data-parallel over batch (B=8) — one batch element per NeuronCore.
Per core, the whole block runs out of SBUF:

  xf (256, 4096) -> theta_T = wt@xf + bt      (128, N)   [PE + DVE bias]
                    phi     = wp@xf + bp      (128, N)   [PE + DVE bias]
                    g0      = (wg@xf)^T       (N, 128)   [PE bf16; bf16 x
                                                          via SWDGE cast-DMA]
  S^T[m, n] = sum_i phi[i,m] * theta_T[i,n]   (keys m on partitions)
  expS = exp(S^T - 40)                         [ACT]
  sums[n] = sum_m expS[m, n]                   [DVE/GP partials + PE fold]
  yu[o, n] = sum_m g0[m,o] expS[m,n]           (unnormalized)
  o[c,n] = (wW @ yu)[c,n] / sums[n] + bW'[c]
  out = o + xf   — via DMA: xf pre-stored to out, o added on top with an
                   SWDGE accumulate-store (CCE per-element add), so the
                   residual costs no compute-engine time.

Softmax runs without a per-row max: scores are ~N(0, 128) with empirical
|S| < ~91, so exp(S - 40) (a global shift — softmax-invariant) stays
inside fp32 range. The normalization is commuted past the wW matmul
(divide after, per-column), so the PSUM->SBUF copy of yu does not wait
on the reciprocal and the next q's matmuls start immediately.

Engine budget per core: PE ~136us (S/y matmuls), ACT ~134us (exp only),
DVE ~120us (bias adds, sum partials, epilogue), GpSimd ~50us (4 sum
chunks per q + SWDGE descriptor generation). Matmuls use float32r
(fp22, 1 PE pass); attention probabilities and g are bf16.
"""

import numpy as np
from contextlib import ExitStack

import concourse.bass as bass
import concourse.mybir as mybir
import concourse.tile as tile
from concourse import bacc

P = 128          # partitions / inter channels
C = 256          # input channels
F32 = mybir.dt.float32
F32R = mybir.dt.float32r
AF = mybir.ActivationFunctionType
ALU = mybir.AluOpType
BF16 = mybir.dt.bfloat16
CSHIFT = 40.0    # global score shift before exp (softmax-invariant)

B_FULL = 8
H_FULL = 64
W_FULL = 64
N_FULL = H_FULL * W_FULL

def build_nc(N=N_FULL, NQ=1024):
    """Build the single-core Bass module (SPMD: same NEFF on all 8 cores)."""
    assert N % 512 == 0 and NQ % 512 == 0 and N % NQ == 0
    MC = N // P                   # number of 128-row key chunks
    NB = NQ // 512                # 512-wide matmul blocks per quarter
    NQn = N // NQ                 # query quarters
    NBLK = N // 512               # 512-col x blocks

    nc = bacc.Bacc("TRN2", target_bir_lowering=False, debug=False)

    x_d = nc.dram_tensor("x", [C, N], F32R, kind="ExternalInput").ap()
    xbf_d = nc.dram_tensor("xbf", [C, N], BF16, kind="ExternalInput").ap()
    # weights host-packed to partition-major [128, 2*128] so DMAs are
    # trivially contiguous (one descriptor per partition)
    wtT_d = nc.dram_tensor("wtT", [P, 2 * P], F32R, kind="ExternalInput").ap()
    wpT_d = nc.dram_tensor("wpT", [P, 2 * P], F32R, kind="ExternalInput").ap()
    wgT_d = nc.dram_tensor("wgT", [P, 2 * P], BF16, kind="ExternalInput").ap()
    wWT_d = nc.dram_tensor("wWT", [P, C], F32R, kind="ExternalInput").ap()
    bt_d = nc.dram_tensor("bt", [P, 1], F32, kind="ExternalInput").ap()
    bp_d = nc.dram_tensor("bp", [P, 1], F32, kind="ExternalInput").ap()
    bWp_d = nc.dram_tensor("bWp", [P, 2], F32, kind="ExternalInput").ap()
    out_d = nc.dram_tensor("out", [C, N], F32R, kind="ExternalOutput").ap()

    x_v = x_d.rearrange("(k p) n -> k p n", p=P)
    xbf_v = xbf_d.rearrange("(k p) n -> k p n", p=P)
    out_v = out_d.rearrange("(k p) n -> k p n", p=P)

    with tile.TileContext(nc) as tc, ExitStack() as ctx:
        const = ctx.enter_context(tc.tile_pool(name="const", bufs=1))
        big = ctx.enter_context(tc.tile_pool(name="big", bufs=1))
        work = ctx.enter_context(tc.tile_pool(name="work", bufs=3))
        ps = ctx.enter_context(tc.tile_pool(name="ps", bufs=2, space="PSUM"))
        psy = ctx.enter_context(tc.tile_pool(name="psy", bufs=1, space="PSUM"))

        # ---- constant loads ----
        wtT_sb = const.tile([P, 2, P], F32R, name="wtT_sb")
        wpT_sb = const.tile([P, 2, P], F32R, name="wpT_sb")
        wgT_sb = const.tile([P, 2, P], BF16, name="wgT_sb")
        wWT_sb = const.tile([P, C], F32R, name="wWT_sb")
        bt_sb = const.tile([P, 1], F32, name="bt_sb")
        bp_sb = const.tile([P, 1], F32, name="bp_sb")
        bWp_sb = const.tile([P, 2], F32, name="bWp_sb")
        ones_sb = const.tile([P, P], BF16, name="ones_sb")
        cshift_sb = const.tile([P, 1], F32, name="cshift_sb")
        nc.vector.memset(cshift_sb[:], -CSHIFT)

        nc.sync.dma_start(wtT_sb[:], wtT_d.rearrange("p (k i) -> p k i", k=2))
        nc.sync.dma_start(wpT_sb[:], wpT_d.rearrange("p (k i) -> p k i", k=2))
        nc.sync.dma_start(wgT_sb[:], wgT_d.rearrange("p (k i) -> p k i", k=2))
        nc.sync.dma_start(wWT_sb[:], wWT_d)
        nc.sync.dma_start(bt_sb[:], bt_d)
        nc.sync.dma_start(bp_sb[:], bp_d)
        nc.sync.dma_start(bWp_sb[:], bWp_d)
        nc.vector.memset(ones_sb[:], 1.0)

        # ~10us of junk matmuls on memset data, sized to span the DMA
        # launch + first-x-block window (~15us): the PE crosses the HAM
        # activity threshold while waiting for data, so the first real
        # projection matmuls and the S->exp chain run at 2.4GHz, not
        # 1.2GHz. The tile holds one "s" slot until it finishes (~17us);
        # the first S matmul only needs the other slot (~16us).
        warm_ps = ps.tile([P, P], F32, tag="s", name="warm_ps")
        for _ in range(150):
            nc.tensor.matmul(warm_ps[:], ones_sb[:], ones_sb[:],
                             start=True, stop=True, skip_group_check=True)


        x_sb = big.tile([P, 2, N], F32R, name="x_sb")
        xbf_sb = big.tile([P, 2, N], BF16, name="xbf_sb")
        th_sb = big.tile([P, N], F32R, name="th_sb")   # theta^T (i, n)
        ph_sb = big.tile([P, N], F32R, name="ph_sb")   # phi (i, m)
        g_sb = big.tile([P, MC, P], BF16, name="g_sb")  # g0 (m_in, m_chunk, o)

        # ---- x load interleaved with th/ph projections, block by block.
        # xbf loads and the g matmuls are deferred behind the whole x
        # stream: x completes ~25% sooner (per-queue DMA bandwidth is the
        # startup limit), the th/ph proj-slot rotation never waits on
        # xbf, and g chunks still land well ahead of their y-matmul
        # consumers (one chunk per ~1.1us exp step).
        for blk in range(NBLK):
            sl = slice(blk * 512, (blk + 1) * 512)
            for k in range(2):
                nc.sync.dma_start(x_sb[:, k, sl], x_v[k, :, sl])

            th_ps = ps.tile([P, 512], F32, tag="proj", name="th_ps")
            nc.tensor.matmul(th_ps[:], wtT_sb[:, 0], x_sb[:, 0, sl],
                             start=True, stop=False)
            nc.tensor.matmul(th_ps[:], wtT_sb[:, 1], x_sb[:, 1, sl],
                             start=False, stop=True)
            nc.vector.tensor_scalar_add(th_sb[:, sl], th_ps[:], bt_sb[:, 0:1])

            ph_ps = ps.tile([P, 512], F32, tag="proj", name="ph_ps")
            nc.tensor.matmul(ph_ps[:], wpT_sb[:, 0], x_sb[:, 0, sl],
                             start=True, stop=False)
            nc.tensor.matmul(ph_ps[:], wpT_sb[:, 1], x_sb[:, 1, sl],
                             start=False, stop=True)
            nc.vector.tensor_scalar_add(ph_sb[:, sl], ph_ps[:], bp_sb[:, 0:1])

        for blk in range(NBLK):
            sl = slice(blk * 512, (blk + 1) * 512)
            for k in range(2):
                nc.sync.dma_start(xbf_sb[:, k, sl], xbf_v[k, :, sl])
            # g chunks for this block (bf16: full-rate 128-col matmuls)
            for j in range(4):
                mc = blk * 4 + j
                msl = slice(mc * P, (mc + 1) * P)
                g_ps = ps.tile([P, P], F32, tag="proj", name="g_ps")
                nc.tensor.matmul(g_ps[:], xbf_sb[:, 0, msl], wgT_sb[:, 0],
                                 start=True, stop=False)
                nc.tensor.matmul(g_ps[:], xbf_sb[:, 1, msl], wgT_sb[:, 1],
                                 start=False, stop=True)
                nc.vector.tensor_copy(g_sb[:, mc], g_ps[:])

        # residual: pre-store x to out for the first 3 quarters (their o
        # is added on top with SWDGE accumulate-stores); the last quarter
        # adds x on DVE and does a plain store to shorten the tail
        NPRE = N - NQ
        for k in range(2):
            nc.sync.dma_start(out_v[k, :, 0:NPRE], x_sb[:, k, 0:NPRE])

        # ---- attention main loop ----
        for q in range(NQn):
            qsl = slice(q * NQ, (q + 1) * NQ)
            y_ps = psy.tile([P, NQ], F32, tag="y", name="y_ps")
            acc = None             # DVE accumulator

            for mc in range(MC):
                msl = slice(mc * P, (mc + 1) * P)
                s_ps = ps.tile([P, NQ], F32, tag="s", name="s_ps")
                for b in range(NB):
                    bsl = slice(b * 512, (b + 1) * 512)
                    nc.tensor.matmul(
                        s_ps[:, bsl], ph_sb[:, msl],
                        th_sb[:, q * NQ + b * 512: q * NQ + (b + 1) * 512],
                        start=True, stop=True)
                exp_sb = work.tile([P, NQ], BF16, tag="exp", bufs=10,
                                   name="exp_sb")
                nc.scalar.activation(exp_sb[:], s_ps[:], AF.Exp,
                                     bias=cshift_sb[:, 0:1])

                for b in range(NB):
                    bsl = slice(b * 512, (b + 1) * 512)
                    nc.tensor.matmul(
                        y_ps[:, bsl], g_sb[:, mc], exp_sb[:, bsl],
                        start=(mc == 0), stop=(mc == MC - 1),
                        skip_group_check=True)

                # column-sum partials on DVE, single accumulator
                # (tensor_tensor runs at 2x on bf16; GpSimd stays idle —
                # it shares an SBUF port with DVE and contention costs
                # more than its offload saves; the 692ns add keeps pace
                # with the 1114ns exp so the chain never falls behind)
                if acc is None:
                    acc = work.tile([P, NQ], BF16, tag="acc", bufs=1,
                                    name="acc_sb")
                    nc.vector.tensor_copy(acc[:], exp_sb[:])
                else:
                    nc.vector.tensor_add(acc[:], acc[:], exp_sb[:])

            # unnormalized y out of PSUM immediately (frees y_ps for q+1;
            # does NOT wait on the sum/reciprocal path)
            yt_sb = work.tile([P, NQ], F32R, tag="yt", bufs=3, name="yt_sb")
            if q == NQn - 1:
                nc.scalar.activation(yt_sb[:], y_ps[:], AF.Identity)
            else:
                nc.vector.tensor_copy(yt_sb[:], y_ps[:])

            # Whole epilogue runs on 1-bank "proj" PSUM tiles (idle during
            # attention) in 512-col halves, so neither the "s" slots (S
            # prefetch) nor the "y" slot (next q's accumulation) is ever
            # held by epilogue work.
            recip_sb = work.tile([P, NQ], F32, tag="recip", bufs=3,
                                 name="recip_sb")
            for b in range(NB):
                bsl = slice(b * 512, (b + 1) * 512)
                sum_ps = ps.tile([P, 512], F32, tag="proj", name="sum_ps")
                nc.tensor.matmul(sum_ps[:], ones_sb[:], acc[:, bsl],
                                 start=True, stop=True,
                                 skip_group_check=True)
                nc.vector.reciprocal_approx_fast(recip_sb[:, bsl], sum_ps[:])

            # o = (wW @ yu) * recip + bW'; the +x rides the accumulate-
            # store except on the last q, where a DVE add + plain store in
            # 512-col pieces shortens the kernel tail.
            last = (q == NQn - 1)
            for h in range(2):
                o_sb = work.tile([P, NQ], F32R, tag="o", bufs=6, name="o_sb")
                for b in range(NB):
                    bsl = slice(b * 512, (b + 1) * 512)
                    wy_ps = ps.tile([P, 512], F32, tag="proj", name="wy_ps")
                    nc.tensor.matmul(
                        wy_ps[:], wWT_sb[:, h * P:(h + 1) * P],
                        yt_sb[:, bsl], start=True, stop=True)
                    nc.vector.tensor_mul(o_sb[:, bsl], wy_ps[:],
                                         recip_sb[:, bsl])
                    nc.vector.tensor_scalar_add(o_sb[:, bsl], o_sb[:, bsl],
                                                bWp_sb[:, h:h + 1])
                    if last:
                        nc.vector.tensor_add(o_sb[:, bsl], o_sb[:, bsl],
                                             x_sb[:, h, q * NQ + b * 512:
                                                  q * NQ + (b + 1) * 512])
                        nc.sync.dma_start(
                            out_v[h, :, q * NQ + b * 512:
                                  q * NQ + (b + 1) * 512], o_sb[:, bsl])
                if not last:
                    nc.gpsimd.dma_start(out_v[h, :, qsl], o_sb[:],
                                        accum_op=ALU.add)

    nc.compile()
    return nc


_CACHE = {}


def _built(key=(N_FULL, 1024)):
    if key not in _CACHE:
        _CACHE[key] = build_nc(*key)
    return _CACHE[key]


def make_in_maps(x, wg, bg, wt, bt, wp, bp, wW, bW):
    """Host-side prep: per-core input dicts (core b <- batch b)."""
    x = np.asarray(x, np.float32)
    B, C_, H, W = x.shape
    N = H * W
    xf = np.ascontiguousarray(x.reshape(B, C_, N))
    wg, bg, wt, bt, wp, bp, wW, bW = [
        np.asarray(a, np.float32) for a in (wg, bg, wt, bt, wp, bp, wW, bW)]
    def pack(w, dt=np.float32):  # (128, C) conv weight -> partition-major lhsT
        return np.ascontiguousarray(
            w.T.reshape(2, P, P).transpose(1, 0, 2).reshape(P, 2 * P)
        ).astype(dt)

    import ml_dtypes
    wtT, wpT = pack(wt), pack(wp)
    wgT = pack(wg, ml_dtypes.bfloat16)
    wWT = np.ascontiguousarray(wW.T)                       # (128, 256)
    bWp = (wW @ bg + bW).astype(np.float32)                # fold bg into bW
    bWp = np.ascontiguousarray(bWp.reshape(2, P).T)        # (128, 2)
    shared = {
        "wtT": wtT, "wpT": wpT, "wgT": wgT, "wWT": wWT,
        "bt": bt.reshape(P, 1).copy(), "bp": bp.reshape(P, 1).copy(),
        "bWp": bWp,
    }
    return [{"x": np.ascontiguousarray(xf[b]),
             "xbf": np.ascontiguousarray(xf[b].astype(ml_dtypes.bfloat16)),
             **shared} for b in range(B)]


def kernel(x, wg, bg, wt, bt, wp, bp, wW, bW):
    from concourse.bass_utils import run_bass_kernel_spmd

    B, C_, H, W = np.asarray(x).shape
    in_maps = make_in_maps(x, wg, bg, wt, bt, wp, bp, wW, bW)
    nc = _built()
    res = run_bass_kernel_spmd(nc, in_maps, core_ids=list(range(B)))
    out = np.stack([res.results[b]["out"] for b in range(B)])
    return out.reshape(B, C_, H, W).astype(np.float32)



# revision 2
# speedup vs baseline: 1.0816x; 1.0816x over previous
"""NonLocalBlock (single-head attention, N=HW=4096, d=128) on 8 trn2 cores.

Sharding: data-parallel over batch (B=8) - one batch element per NeuronCore.

Per core (vs. the previous 192us version; target ~165us):
  - x is loaded ONCE as fp16 (2MB instead of 6MB), split over 3 DMA queues
    (sync/gpsimd/scalar) so the first blocks land ~8us in.
  - theta/phi/g weights and activations are fp16: S and y matmuls both run
    at full PE rate (fp32 "HIGH" matmuls stream at 2 cyc/col - the old
    f32r path was the EXP-chain pacer).
  - phi's bias is dropped entirely: softmax over keys m is invariant to
    the (theta_n+bt).bp term, so S = (theta+bt).phi exactly.
  - theta is scaled by A=2^7*log2(e) at extraction (tensor_scalar fused
    add+mult); the EXP activation un-scales via its free scale immediate.
    (Prep for a Schraudolph-on-DVE exp offload; harmless otherwise.)
  - Normalization is commuted BEFORE the wW projection: ytn = (yu*recip)
    in fp16, so the epilogue is yt-copy, sums-matmul, recip, one 1024-wide
    mul, 4 wy matmuls, 2 copies, 2 accumulate-DMAs. All epilogue PSUM
    lives in a dedicated 2-bank "wy" tag so the S-slot rotation (2x2
    banks) and the y accumulator (2 banks) never stall at q boundaries.
  - The residual (+ bW' broadcast, folded on host into xpb) is pre-stored
    to out by a single DRAM->DRAM DMA; every quarter's output rides a
    SWDGE accumulate-store.
  - y matmuls are emitted with a 2-chunk lag so a late g chunk can never
    head-of-line-block the S matmuls on the in-order PE queue.
"""

import numpy as np
from contextlib import ExitStack

import concourse.bass as bass
import concourse.mybir as mybir
import concourse.tile as tile
from concourse import bacc

P = 128          # partitions / inter channels
C = 256          # input channels
F32 = mybir.dt.float32
F32R = mybir.dt.float32r
F16 = mybir.dt.float16
BF16 = mybir.dt.bfloat16
AF = mybir.ActivationFunctionType
ALU = mybir.AluOpType
CSHIFT = 40.0    # global score shift before exp (softmax-invariant)
A_SCALE = 184.66496414625282  # 2^7 * log2(e); theta pre-scale

B_FULL = 8
H_FULL = 64
W_FULL = 64
N_FULL = H_FULL * W_FULL

WARM_MMS = 24    # junk matmuls to trip the PE HAM to 8/8 before real work


def build_nc(N=N_FULL, NQ=1024):
    """Build the single-core Bass module (SPMD: same NEFF on all 8 cores)."""
    MC = N // P                   # 32 key chunks
    NQn = N // NQ                 # 4 query quarters
    NB = NQ // 512                # 2 512-col blocks per quarter
    NCB = N // 1024               # 4 1024-col proj blocks

    nc = bacc.Bacc("TRN2", target_bir_lowering=False, debug=False)

    xh_d = nc.dram_tensor("xh", [C, N], F16, kind="ExternalInput").ap()
    xpb_d = nc.dram_tensor("xpb", [C, N], F32R, kind="ExternalInput").ap()
    wtT_d = nc.dram_tensor("wtT", [P, 2 * P], F16, kind="ExternalInput").ap()
    wpT_d = nc.dram_tensor("wpT", [P, 2 * P], F16, kind="ExternalInput").ap()
    wgT_d = nc.dram_tensor("wgT", [P, 2 * P], F16, kind="ExternalInput").ap()
    wWT_d = nc.dram_tensor("wWT", [P, C], F16, kind="ExternalInput").ap()
    bt_d = nc.dram_tensor("bt", [P, 1], F32, kind="ExternalInput").ap()
    out_d = nc.dram_tensor("out", [C, N], F32R, kind="ExternalOutput").ap()

    xh_v = xh_d.rearrange("(k p) n -> k p n", p=P)
    out_v = out_d.rearrange("(k p) n -> k p n", p=P)

    with tile.TileContext(nc) as tc, ExitStack() as ctx:
        const = ctx.enter_context(tc.tile_pool(name="const", bufs=1))
        big = ctx.enter_context(tc.tile_pool(name="big", bufs=1))
        work = ctx.enter_context(tc.tile_pool(name="work", bufs=1))
        ps = ctx.enter_context(tc.tile_pool(name="ps", bufs=1, space="PSUM"))

        # ---- constants ----
        wtT_sb = const.tile([P, 2, P], F16, name="wtT_sb")
        wpT_sb = const.tile([P, 2, P], F16, name="wpT_sb")
        wgT_sb = const.tile([P, 2, P], F16, name="wgT_sb")
        wWT_sb = const.tile([P, C], F16, name="wWT_sb")
        bt_sb = const.tile([P, 1], F32, name="bt_sb")
        ones_sb = const.tile([P, P], BF16, name="ones_sb")
        cshift_sb = const.tile([P, 1], F32, name="cshift_sb")
        nc.vector.memset(cshift_sb[:], -CSHIFT)
        nc.vector.memset(ones_sb[:], 1.0)

        # weights on the scalar queue (small; lands before first use)
        nc.scalar.dma_start(wtT_sb[:], wtT_d.rearrange("p (k i) -> p k i", k=2))
        nc.scalar.dma_start(wpT_sb[:], wpT_d.rearrange("p (k i) -> p k i", k=2))
        nc.scalar.dma_start(bt_sb[:], bt_d)
        nc.scalar.dma_start(wgT_sb[:], wgT_d.rearrange("p (k i) -> p k i", k=2))
        nc.scalar.dma_start(wWT_sb[:], wWT_d)

        # x halves on two parallel queues: k=0 via sync, k=1 via gpsimd
        xh_sb = big.tile([P, 2, N], F16, name="xh_sb")
        for b in range(8):
            sl = slice(b * 512, (b + 1) * 512)
            nc.sync.dma_start(xh_sb[:, 0, sl], xh_v[0, :, sl])
        for b in range(8):
            sl = slice(b * 512, (b + 1) * 512)
            nc.gpsimd.dma_start(xh_sb[:, 1, sl], xh_v[1, :, sl])

        # residual (+bW', folded on host) pre-store: one DRAM->DRAM copy,
        # queued behind the x stream on the sync queue.
        nc.sync.dma_start(out_d, xpb_d)

        th_sb = big.tile([P, N], F16, name="th_sb")   # A*(theta+bt), (i, n)
        ph_sb = big.tile([P, N], F16, name="ph_sb")   # phi (no bias), (i, m)
        g_sb = big.tile([P, MC, P], BF16, name="g_sb")  # g (m_in, mc, o)

        # ---- PE warmup: trip HAM to 8/8 while the first x blocks land ----
        warm_ps = ps.tile([P, P], F32, tag="s", bufs=2, name="warm_ps")
        for _ in range(WARM_MMS):
            nc.tensor.matmul(warm_ps[:], ones_sb[:], ones_sb[:],
                             start=True, stop=True, skip_group_check=True)

        def proj_block(dst, w_sb, cb, tag, engine):
            """theta/phi projection for 1024-col block cb; extract on
            `engine` ('act'|'dve'); theta also gets (+bt)*A fused."""
            sl = slice(cb * 1024, (cb + 1) * 1024)
            p_ps = ps.tile([P, 1024], F32, tag=tag, bufs=None if tag == "wy"
                           else 2, name="p_ps")
            for h in range(2):
                hsl = slice(cb * 1024 + h * 512, cb * 1024 + (h + 1) * 512)
                for k in range(2):
                    nc.tensor.matmul(p_ps[:, h * 512:(h + 1) * 512],
                                     w_sb[:, k], xh_sb[:, k, hsl],
                                     start=(k == 0), stop=(k == 1))
            if dst is th_sb:
                nc.vector.tensor_scalar(th_sb[:, sl], p_ps[:],
                                        bt_sb[:, 0:1], A_SCALE,
                                        op0=ALU.add, op1=ALU.mult)
            elif engine == "act":
                nc.scalar.activation(ph_sb[:, sl], p_ps[:], AF.Identity)
            else:
                nc.vector.tensor_copy(ph_sb[:, sl], p_ps[:])

        def g_block(t):
            """g for key chunks 4t..4t+3 (x block t)."""
            g_ps = ps.tile([P, 4, P], F32, tag="wy", name="g_ps")
            for j in range(4):
                msl = slice((4 * t + j) * P, (4 * t + j + 1) * P)
                for k in range(2):
                    nc.tensor.matmul(g_ps[:, j], xh_sb[:, k, msl],
                                     wgT_sb[:, k], start=(k == 0),
                                     stop=(k == 1))
            nc.vector.tensor_copy(g_sb[:, 4 * t:4 * t + 4], g_ps[:])

        # critical first blocks: th0 (extract on DVE) + ph0 (extract on ACT)
        # run in the "s" slots before the attention chain starts; g0 too so
        # the first y matmuls never stall the in-order PE queue.
        proj_block(th_sb, wtT_sb, 0, "s", "dve")
        proj_block(ph_sb, wpT_sb, 0, "s", "act")
        g_block(0)

        # deferred prep, interleaved into q0's chunk loop (emission order
        # == PE queue order; all x-arrival times comfortably precede these)
        prep = {1: lambda: g_block(1),
                3: lambda: proj_block(ph_sb, wpT_sb, 1, "wy", "dve"),
                5: lambda: g_block(2),
                7: lambda: g_block(3),
                9: lambda: proj_block(ph_sb, wpT_sb, 2, "wy", "dve"),
                11: lambda: g_block(4),
                13: lambda: g_block(5),
                15: lambda: proj_block(ph_sb, wpT_sb, 3, "wy", "dve"),
                17: lambda: g_block(6),
                19: lambda: g_block(7),
                21: lambda: proj_block(th_sb, wtT_sb, 1, "wy", "dve"),
                23: lambda: proj_block(th_sb, wtT_sb, 2, "wy", "dve"),
                25: lambda: proj_block(th_sb, wtT_sb, 3, "wy", "dve")}

        YLAG = 2  # y-matmul emission lag (chunks), protects PE queue order

        for q in range(NQn):
            qsl = slice(q * NQ, (q + 1) * NQ)
            y_ps = ps.tile([P, NQ], F32, tag="y", name="y_ps")
            acc = work.tile([P, NQ], BF16, tag="acc", bufs=2, name="acc_sb")
            exps = {}

            def y_mms(mc):
                e = exps.pop(mc)
                for b in range(NB):
                    bsl = slice(b * 512, (b + 1) * 512)
                    nc.tensor.matmul(y_ps[:, bsl], g_sb[:, mc], e[:, bsl],
                                     start=(mc == 0), stop=(mc == MC - 1),
                                     skip_group_check=True)

            for mc in range(MC):
                s_ps = ps.tile([P, NQ], F32, tag="s", bufs=2, name="s_ps")
                msl = slice(mc * P, (mc + 1) * P)
                for b in range(NB):
                    bsl = slice(b * 512, (b + 1) * 512)
                    nc.tensor.matmul(
                        s_ps[:, bsl], ph_sb[:, msl],
                        th_sb[:, q * NQ + b * 512: q * NQ + (b + 1) * 512],
                        start=True, stop=True)
                exp_sb = work.tile([P, NQ], BF16, tag="exp", bufs=12,
                                   name="exp_sb")
                nc.scalar.activation(exp_sb[:], s_ps[:], AF.Exp,
                                     bias=cshift_sb[:, 0:1],
                                     scale=1.0 / A_SCALE)
                exps[mc] = exp_sb
                if mc == 0:
                    nc.vector.tensor_copy(acc[:], exp_sb[:])
                else:
                    nc.vector.tensor_add(acc[:], acc[:], exp_sb[:])
                if mc >= YLAG:
                    y_mms(mc - YLAG)
                if q == 0 and mc in prep:
                    prep[mc]()
            for mc in range(MC - YLAG, MC):
                y_mms(mc)

            # ---- epilogue: all PSUM on the dedicated "wy" tag ----
            yt_sb = work.tile([P, NQ], F32R, tag="yt", bufs=2, name="yt_sb")
            nc.vector.tensor_copy(yt_sb[:], y_ps[:])   # frees y for q+1

            sum_ps = ps.tile([P, NQ], F32, tag="wy", name="sum_ps")
            for b in range(NB):
                bsl = slice(b * 512, (b + 1) * 512)
                nc.tensor.matmul(sum_ps[:, bsl], ones_sb[:], acc[:, bsl],
                                 start=True, stop=True,
                                 skip_group_check=True)
            recip_sb = work.tile([P, NQ], F32, tag="recip", bufs=2,
                                 name="recip_sb")
            nc.vector.reciprocal_approx_fast(recip_sb[:], sum_ps[:])
            ytn_sb = work.tile([P, NQ], F16, tag="ytn", bufs=2, name="ytn_sb")
            nc.vector.tensor_mul(ytn_sb[:], yt_sb[:], recip_sb[:])

            for h in range(2):
                wy_ps = ps.tile([P, NQ], F32, tag="wy", name="wy_ps")
                for b in range(NB):
                    bsl = slice(b * 512, (b + 1) * 512)
                    nc.tensor.matmul(wy_ps[:, bsl],
                                     wWT_sb[:, h * P:(h + 1) * P],
                                     ytn_sb[:, bsl], start=True, stop=True)
                o_sb = work.tile([P, NQ], F32R, tag="o", bufs=2, name="o_sb")
                nc.vector.tensor_copy(o_sb[:], wy_ps[:])
                nc.gpsimd.dma_start(out_v[h, :, qsl], o_sb[:],
                                    accum_op=ALU.add)

    nc.compile()
    return nc


_CACHE = {}


def _built(key=(N_FULL, 1024)):
    if key not in _CACHE:
        _CACHE[key] = build_nc(*key)
    return _CACHE[key]


def make_in_maps(x, wg, bg, wt, bt, wp, bp, wW, bW):
    """Host-side prep: per-core input dicts (core b <- batch b)."""
    x = np.asarray(x, np.float32)
    B, C_, H, W = x.shape
    N = H * W
    xf = np.ascontiguousarray(x.reshape(B, C_, N))
    wg, bg, wt, bt, wp, bp, wW, bW = [
        np.asarray(a, np.float32) for a in (wg, bg, wt, bt, wp, bp, wW, bW)]

    def pack(w):  # (128, C) conv weight -> partition-major lhsT, fp16
        return np.ascontiguousarray(
            w.T.reshape(2, P, P).transpose(1, 0, 2).reshape(P, 2 * P)
        ).astype(np.float16)

    bWp = (wW @ bg + bW).astype(np.float32)       # fold bg into bW
    shared = {
        "wtT": pack(wt), "wpT": pack(wp), "wgT": pack(wg),
        "wWT": np.ascontiguousarray(wW.T).astype(np.float16),
        "bt": bt.reshape(P, 1).copy(),
    }
    return [{"xh": np.ascontiguousarray(xf[b]).astype(np.float16),
             "xpb": np.ascontiguousarray(xf[b] + bWp[:, None]),
             **shared} for b in range(B)]


def kernel(x, wg, bg, wt, bt, wp, bp, wW, bW):
    from concourse.bass_utils import run_bass_kernel_spmd

    B, C_, H, W = np.asarray(x).shape
    in_maps = make_in_maps(x, wg, bg, wt, bt, wp, bp, wW, bW)
    nc = _built()
    res = run_bass_kernel_spmd(nc, in_maps, core_ids=list(range(B)))
    out = np.stack([res.results[b]["out"] for b in range(B)])
    return out.reshape(B, C_, H, W).astype(np.float32)


# revision 5
# speedup vs baseline: 1.0870x; 1.0050x over previous
"""NonLocalBlock (single-head attention, N=HW=4096, d=128) on 8 trn2 cores.

Sharding: data-parallel over batch (B=8) - one batch element per NeuronCore.

Design notes (vs. the 192us baseline):
  - x is loaded ONCE as fp16 (2MB instead of 6MB), split over 3 DMA queues.
  - theta/phi/g weights and activations are fp16: S and y matmuls both run
    at full PE rate (the old f32r S matmuls streamed at 2 cyc/col and were
    the EXP-chain pacer).
  - phi's bias is dropped (softmax over keys is invariant to it); theta's
    bias+scale are fused into its PSUM extraction.
  - theta carries A=2^7*log2(e) so selected chunks can compute exp on the
    DVE via the Schraudolph bit trick: i16 = clamp(S' + B, 0) reinterpreted
    as bf16 IS e^(S-40) to ~2%. One tensor_scalar (add,max) per chunk, so
    5 of 32 chunks per quarter come off the Scalar engine's critical path.
  - theta/phi/g live in PER-BLOCK tiles: matmul weight (lhsT) reads get
    conservative (whole-tile) dependencies, so a shared tile would stall
    early S/y matmuls on unrelated later extracts (measured +8us).
  - Normalization is commuted BEFORE the wW projection (ytn = yu*recip in
    fp16), so each quarter's output is extracted once and accumulated onto
    the pre-stored residual (x + bW', folded on host, one DRAM->DRAM DMA).
  - The LAST quarter uses plain stores (residual added on DVE from a
    pre-loaded xpb slice): a trailing SWDGE accumulate costs ~7us in
    CCE + drain at the ramped-down tail.
  - y matmuls are emitted with a 2-chunk lag so a late g chunk can never
    head-of-line-block the S matmuls on the in-order PE queue.
"""

import numpy as np
from contextlib import ExitStack

import concourse.bass as bass
import concourse.mybir as mybir
import concourse.tile as tile
from concourse import bacc

P = 128          # partitions / inter channels
C = 256          # input channels
F32 = mybir.dt.float32
F32R = mybir.dt.float32r
F16 = mybir.dt.float16
BF16 = mybir.dt.bfloat16
I16 = mybir.dt.int16
AF = mybir.ActivationFunctionType
ALU = mybir.AluOpType
CSHIFT = 40.0    # global score shift before exp (softmax-invariant)
A_SCALE = 184.66496414625282          # 2^7 * log2(e); theta pre-scale
B_SHIFT = 16256.0 - 5.5 - CSHIFT * A_SCALE  # Schraudolph offset (C=5.5)

B_FULL = 8
H_FULL = 64
W_FULL = 64
N_FULL = H_FULL * W_FULL

WARM_MMS = 40    # junk matmuls to trip the PE HAM to 8/8 before real work
DVE_EXP = {5, 11, 17, 23, 29}   # chunks whose exp runs on DVE (q>=1)


def build_nc(N=N_FULL, NQ=1024):
    """Build the single-core Bass module (SPMD: same NEFF on all 8 cores)."""
    MC = N // P                   # 32 key chunks
    NQn = N // NQ                 # 4 query quarters
    NB = NQ // 512                # 2 512-col blocks per quarter
    NCB = N // 1024               # 4 1024-col proj blocks

    nc = bacc.Bacc("TRN2", target_bir_lowering=False, debug=False)

    xh_d = nc.dram_tensor("xh", [C, N], F16, kind="ExternalInput").ap()
    xpb_d = nc.dram_tensor("xpb", [C, N], F32R, kind="ExternalInput").ap()
    wtT_d = nc.dram_tensor("wtT", [P, 2 * P], F16, kind="ExternalInput").ap()
    wpT_d = nc.dram_tensor("wpT", [P, 2 * P], F16, kind="ExternalInput").ap()
    wgT_d = nc.dram_tensor("wgT", [P, 2 * P], F16, kind="ExternalInput").ap()
    wWT_d = nc.dram_tensor("wWT", [P, C], F16, kind="ExternalInput").ap()
    bt_d = nc.dram_tensor("bt", [P, 1], F32, kind="ExternalInput").ap()
    out_d = nc.dram_tensor("out", [C, N], F32R, kind="ExternalOutput").ap()

    xh_v = xh_d.rearrange("(k p) n -> k p n", p=P)
    xpb_v = xpb_d.rearrange("(k p) n -> k p n", p=P)
    out_v = out_d.rearrange("(k p) n -> k p n", p=P)

    with tile.TileContext(nc) as tc, ExitStack() as ctx:
        const = ctx.enter_context(tc.tile_pool(name="const", bufs=1))
        big = ctx.enter_context(tc.tile_pool(name="big", bufs=1))
        work = ctx.enter_context(tc.tile_pool(name="work", bufs=1))
        ps = ctx.enter_context(tc.tile_pool(name="ps", bufs=1, space="PSUM"))

        # ---- constants ----
        wtT_sb = const.tile([P, 2, P], F16, name="wtT_sb")
        wpT_sb = const.tile([P, 2, P], F16, name="wpT_sb")
        wgT_sb = const.tile([P, 2, P], F16, name="wgT_sb")
        wWT_sb = const.tile([P, C], F16, name="wWT_sb")
        bt_sb = const.tile([P, 1], F32, name="bt_sb")
        ones_sb = const.tile([P, P], BF16, name="ones_sb")
        cshift_sb = const.tile([P, 1], F32, name="cshift_sb")
        nc.vector.memset(cshift_sb[:], -CSHIFT)
        nc.vector.memset(ones_sb[:], 1.0)

        # weights on the scalar queue (small; lands before first use)
        nc.scalar.dma_start(wtT_sb[:], wtT_d.rearrange("p (k i) -> p k i", k=2))
        nc.scalar.dma_start(wpT_sb[:], wpT_d.rearrange("p (k i) -> p k i", k=2))
        nc.scalar.dma_start(bt_sb[:], bt_d)
        nc.scalar.dma_start(wgT_sb[:], wgT_d.rearrange("p (k i) -> p k i", k=2))
        nc.scalar.dma_start(wWT_sb[:], wWT_d)

        # x halves on two parallel queues: k=0 via sync, k=1 via gpsimd
        xh_sb = big.tile([P, 2, N], F16, name="xh_sb")
        for b in range(8):
            sl = slice(b * 512, (b + 1) * 512)
            nc.sync.dma_start(xh_sb[:, 0, sl], xh_v[0, :, sl])
        for b in range(8):
            sl = slice(b * 512, (b + 1) * 512)
            nc.gpsimd.dma_start(xh_sb[:, 1, sl], xh_v[1, :, sl])

        # residual (+bW', folded on host) pre-store: one DRAM->DRAM copy,
        # queued behind the x stream on the sync queue. The LAST quarter is
        # stored plainly instead, with its residual slice added from SBUF.
        NPRE = N - NQ
        nc.sync.dma_start(out_d[:, 0:NPRE], xpb_d[:, 0:NPRE])
        xpbq_sb = big.tile([P, 2, NQ], F32R, name="xpbq_sb")
        for k in range(2):
            nc.sync.dma_start(xpbq_sb[:, k], xpb_v[k, :, NPRE:N])

        # per-block tiles (single writer each -> exact matmul weight deps)
        th_t = [big.tile([P, 1024], F16, name=f"th{i}") for i in range(NCB)]
        ph_t = [big.tile([P, 1024], F16, name=f"ph{i}") for i in range(NCB)]
        g_t = [big.tile([P, 4, P], BF16, name=f"g{i}") for i in range(8)]

        # ---- PE warmup: trip HAM to 8/8 while the first x blocks land ----
        warm_ps = ps.tile([P, P], F32, tag="s", bufs=2, name="warm_ps")
        for _ in range(WARM_MMS):
            nc.tensor.matmul(warm_ps[:], ones_sb[:], ones_sb[:],
                             start=True, stop=True, skip_group_check=True)

        def proj_block(dst_t, w_sb, cb, tag, engine):
            """theta/phi projection for 1024-col block cb; extract on
            `engine` ('act'|'dve'); theta also gets (+bt)*A fused."""
            is_th = dst_t is th_t
            p_ps = ps.tile([P, 1024], F32, tag=tag, bufs=2 if tag == "s"
                           else None, name="p_ps")
            for h in range(2):
                hsl = slice(cb * 1024 + h * 512, cb * 1024 + (h + 1) * 512)
                for k in range(2):
                    nc.tensor.matmul(p_ps[:, h * 512:(h + 1) * 512],
                                     w_sb[:, k], xh_sb[:, k, hsl],
                                     start=(k == 0), stop=(k == 1))
            if is_th:
                nc.vector.tensor_scalar(dst_t[cb][:], p_ps[:],
                                        bt_sb[:, 0:1], A_SCALE,
                                        op0=ALU.add, op1=ALU.mult)
            elif engine == "act":
                nc.scalar.activation(dst_t[cb][:], p_ps[:], AF.Identity)
            else:
                nc.vector.tensor_copy(dst_t[cb][:], p_ps[:])

        def g_block(t):
            """g for key chunks 4t..4t+3 (x block t)."""
            g_ps = ps.tile([P, 4, P], F32, tag="wy", name="g_ps")
            for j in range(4):
                msl = slice((4 * t + j) * P, (4 * t + j + 1) * P)
                for k in range(2):
                    nc.tensor.matmul(g_ps[:, j], xh_sb[:, k, msl],
                                     wgT_sb[:, k], start=(k == 0),
                                     stop=(k == 1))
            nc.vector.tensor_copy(g_t[t][:], g_ps[:])

        # critical first blocks: th0 (extract on DVE) + ph0 (extract on ACT)
        # run in the "s" slots before the attention chain starts; g0 too so
        # the first y matmuls never stall the in-order PE queue.
        proj_block(th_t, wtT_sb, 0, "s", "dve")
        proj_block(ph_t, wpT_sb, 0, "s", "act")
        g_block(0)

        # deferred prep, interleaved into q0's chunk loop (emission order
        # == PE queue order; x-arrival times comfortably precede these)
        prep = {1: lambda: g_block(1),
                3: lambda: proj_block(ph_t, wpT_sb, 1, "wy", "dve"),
                5: lambda: g_block(2),
                7: lambda: g_block(3),
                9: lambda: proj_block(ph_t, wpT_sb, 2, "wy", "dve"),
                11: lambda: g_block(4),
                13: lambda: g_block(5),
                15: lambda: proj_block(ph_t, wpT_sb, 3, "wy", "dve"),
                17: lambda: g_block(6),
                19: lambda: g_block(7),
                21: lambda: proj_block(th_t, wtT_sb, 1, "wy", "dve"),
                23: lambda: proj_block(th_t, wtT_sb, 2, "wy", "dve"),
                25: lambda: proj_block(th_t, wtT_sb, 3, "wy", "dve")}

        YLAG = 2  # y-matmul emission lag (chunks), protects PE queue order

        for q in range(NQn):
            qsl = slice(q * NQ, (q + 1) * NQ)
            last = (q == NQn - 1)
            y_ps = ps.tile([P, NQ], F32, tag="y", name="y_ps")
            acc = work.tile([P, NQ], BF16, tag="acc", bufs=2, name="acc_sb")
            exps = {}

            def y_mms(mc):
                e = exps.pop(mc)
                for b in range(NB):
                    bsl = slice(b * 512, (b + 1) * 512)
                    nc.tensor.matmul(y_ps[:, bsl], g_t[mc // 4][:, mc % 4],
                                     e[:, bsl],
                                     start=(mc == 0), stop=(mc == MC - 1),
                                     skip_group_check=True)

            for mc in range(MC):
                s_ps = ps.tile([P, NQ], F32, tag="s", bufs=2, name="s_ps")
                msl = slice((mc % 8) * P, (mc % 8 + 1) * P)
                for b in range(NB):
                    bsl = slice(b * 512, (b + 1) * 512)
                    nc.tensor.matmul(
                        s_ps[:, bsl], ph_t[mc // 8][:, msl],
                        th_t[q][:, b * 512:(b + 1) * 512],
                        start=True, stop=True)
                exp_sb = work.tile([P, NQ], BF16, tag="exp", bufs=12,
                                   name="exp_sb")
                if q > 0 and mc in DVE_EXP:
                    # Schraudolph: bf16 bits of e^(S-40) via one DVE op
                    nc.vector.tensor_scalar(exp_sb[:].bitcast(I16), s_ps[:],
                                            B_SHIFT, 0.0,
                                            op0=ALU.add, op1=ALU.max)
                else:
                    nc.scalar.activation(exp_sb[:], s_ps[:], AF.Exp,
                                         bias=cshift_sb[:, 0:1],
                                         scale=1.0 / A_SCALE)
                exps[mc] = exp_sb
                if mc == 0:
                    nc.vector.tensor_copy(acc[:], exp_sb[:])
                else:
                    nc.vector.tensor_add(acc[:], acc[:], exp_sb[:])
                if mc >= YLAG:
                    y_mms(mc - YLAG)
                if q == 0 and mc in prep:
                    prep[mc]()
            for mc in range(MC - YLAG, MC):
                y_mms(mc)

            # ---- epilogue: all PSUM on the dedicated "wy" tag ----
            yt_sb = work.tile([P, NQ], F32R, tag="yt", bufs=2, name="yt_sb")
            if last:
                nc.scalar.activation(yt_sb[:], y_ps[:], AF.Identity)
            else:
                nc.vector.tensor_copy(yt_sb[:], y_ps[:])

            sum_ps = ps.tile([P, NQ], F32, tag="wy", name="sum_ps")
            for b in range(NB):
                bsl = slice(b * 512, (b + 1) * 512)
                nc.tensor.matmul(sum_ps[:, bsl], ones_sb[:], acc[:, bsl],
                                 start=True, stop=True,
                                 skip_group_check=True)
            recip_sb = work.tile([P, NQ], F32, tag="recip", bufs=2,
                                 name="recip_sb")
            ytn_sb = work.tile([P, NQ], F16, tag="ytn", bufs=2, name="ytn_sb")
            if last:   # 512-wide pieces pipeline with the wy matmuls
                for b in range(NB):
                    bsl = slice(b * 512, (b + 1) * 512)
                    nc.vector.reciprocal_approx_fast(recip_sb[:, bsl],
                                                     sum_ps[:, bsl])
                    nc.vector.tensor_mul(ytn_sb[:, bsl], yt_sb[:, bsl],
                                         recip_sb[:, bsl])
            else:
                nc.vector.reciprocal_approx_fast(recip_sb[:], sum_ps[:])
                nc.vector.tensor_mul(ytn_sb[:], yt_sb[:], recip_sb[:])

            for h in range(2):
                wy_ps = ps.tile([P, NQ], F32, tag="wy", name="wy_ps")
                for b in range(NB):
                    bsl = slice(b * 512, (b + 1) * 512)
                    nc.tensor.matmul(wy_ps[:, bsl],
                                     wWT_sb[:, h * P:(h + 1) * P],
                                     ytn_sb[:, bsl], start=True, stop=True)
                if last:
                    for b in range(NB):
                        bsl = slice(b * 512, (b + 1) * 512)
                        o_sb = work.tile([P, 512], F32R, tag="o", bufs=4,
                                         name="o_sb")
                        nc.vector.tensor_add(o_sb[:], wy_ps[:, bsl],
                                             xpbq_sb[:, h, bsl])
                        nc.sync.dma_start(
                            out_v[h, :, q * NQ + b * 512:
                                  q * NQ + (b + 1) * 512], o_sb[:])
                else:
                    o_sb = work.tile([P, NQ], F32R, tag="ow", bufs=2,
                                     name="o_sb")
                    nc.vector.tensor_copy(o_sb[:], wy_ps[:])
                    nc.gpsimd.dma_start(out_v[h, :, qsl], o_sb[:],
                                        accum_op=ALU.add)

    nc.compile()
    return nc


_CACHE = {}


def _built(key=(N_FULL, 1024)):
    if key not in _CACHE:
        _CACHE[key] = build_nc(*key)
    return _CACHE[key]


def make_in_maps(x, wg, bg, wt, bt, wp, bp, wW, bW):
    """Host-side prep: per-core input dicts (core b <- batch b)."""
    x = np.asarray(x, np.float32)
    B, C_, H, W = x.shape
    N = H * W
    xf = np.ascontiguousarray(x.reshape(B, C_, N))
    wg, bg, wt, bt, wp, bp, wW, bW = [
        np.asarray(a, np.float32) for a in (wg, bg, wt, bt, wp, bp, wW, bW)]

    def pack(w):  # (128, C) conv weight -> partition-major lhsT, fp16
        return np.ascontiguousarray(
            w.T.reshape(2, P, P).transpose(1, 0, 2).reshape(P, 2 * P)
        ).astype(np.float16)

    bWp = (wW @ bg + bW).astype(np.float32)       # fold bg into bW
    shared = {
        "wtT": pack(wt), "wpT": pack(wp), "wgT": pack(wg),
        "wWT": np.ascontiguousarray(wW.T).astype(np.float16),
        "bt": bt.reshape(P, 1).copy(),
    }
    return [{"xh": np.ascontiguousarray(xf[b]).astype(np.float16),
             "xpb": np.ascontiguousarray(xf[b] + bWp[:, None]),
             **shared} for b in range(B)]


def kernel(x, wg, bg, wt, bt, wp, bp, wW, bW):
    from concourse.bass_utils import run_bass_kernel_spmd

    B, C_, H, W = np.asarray(x).shape
    in_maps = make_in_maps(x, wg, bg, wt, bt, wp, bp, wW, bW)
    nc = _built()
    res = run_bass_kernel_spmd(nc, in_maps, core_ids=list(range(B)))
    out = np.stack([res.results[b]["out"] for b in range(B)])
    return out.reshape(B, C_, H, W).astype(np.float32)
